# revision 1
# baseline (speedup 1.0000x reference)
"""Trainium2 Bass kernel for nn_DDC2Loss: mean of strict-upper-triangle of A@A.T.

Identity: sum_{i<j} <a_i,a_j> = (||colsum(A)||^2 - sum(A*A)) / 2, so the
kernel only needs a column-sum partial and a sum-of-squares partial per
row-shard; the tiny final combine runs on host in float64.

Data-parallel over rows: each of 8 cores gets a (2048, 512) shard and
returns out_cs [128,512] (per-partition column-sum partial) and
out_sq [128,9] (per-partition sum-of-squares partials).

Raw bass (no Tile). Per core: SP issues 9 input DMAs (7 big chunks with
4KB contiguous runs + 2 small trailing chunks) plus the out_cs DMA; DVE
runs a 1024-wide add chain ending in a single post-stream fold; ACT does
Square+accum_out for all chunks and ships out_sq itself; GpSimd seeds the
bias const and waits for outputs. A post-build pass strips the unused
const-AP memsets and the entry all-engine barrier; the NEFF epilogue's
own semaphore teardown (after the Block-exit barrier) restores sem state
for repeat executions.
"""

import os
import sys

import numpy as np

for _p in (
    "/root/.axon_site",
    "/root/.axon_site/_ro/trn_rl_repo",
    "/root/.axon_site/_ro/pypackages",
    "/opt/trn_rl_repo",
):
    if os.path.isdir(_p) and _p not in sys.path:
        sys.path.append(_p)

from concourse.bass_utils import run_bass_kernel_spmd


def _install_ntff_shim():
    """This image's antenv lacks axon_hooks, but bass_utils imports it when
    BASS_TRACE is set. Synthesize the module (wired to the ctypes NTFF
    profiler from trn_agent_boot when available) so tracing works instead
    of crashing."""
    import types

    if "antenv.axon_hooks" in sys.modules:
        return
    try:
        import antenv  # noqa: F401
    except Exception:
        return
    if getattr(antenv, "axon_hooks", None) is not None:
        return
    mod = types.ModuleType("antenv.axon_hooks")
    mod._hook = None

    def set_axon_ntff_profile_hook(h):
        mod._hook = h

    def get_axon_ntff_profile_hook():
        return mod._hook

    mod.set_axon_ntff_profile_hook = set_axon_ntff_profile_hook
    mod.get_axon_ntff_profile_hook = get_axon_ntff_profile_hook
    sys.modules["antenv.axon_hooks"] = mod
    antenv.axon_hooks = mod
    try:
        from trn_agent_boot.trn_boot import _ntff_profile_via_ctypes

        so = "/opt/axon/libaxon_pjrt.so"
        if os.path.exists(so):
            mod._hook = _ntff_profile_via_ctypes(so)
        import concourse.bass_utils as _bu

        _orig_upload = _bu.upload_artifacts

        def _safe_upload(tmpdir):
            try:
                return _orig_upload(tmpdir)
            except Exception:
                return tmpdir

        _bu.upload_artifacts = _safe_upload
    except Exception:
        pass


_install_ntff_shim()

from contextlib import ExitStack

import concourse.bass as bass
import concourse.mybir as mybir

N_CORES = 8
N_ROWS = 16384
N_COLS = 512
SHARD_ROWS = N_ROWS // N_CORES  # 2048
P = 128
N_TILES = SHARD_ROWS // P  # 16
NBIG = 7  # big chunks (2 tiles each)
N_CHUNKS = 9  # 7 big + 2 small
N_STAT = 9  # stats columns, one per square op

F32 = mybir.dt.float32


def _strip_entry_overhead(nc):
    """Remove the const-AP memsets and the entry all-engine barrier from the
    first block; this kernel uses neither (bias is an explicit tile)."""
    main = nc.m.functions[0].blocks[0]
    keep = []
    removed = []
    for inst in main.instructions:
        kind = type(inst).__name__
        drop = False
        if kind == "InstDrain":
            drop = True
        elif kind == "InstRegisterMove":
            drop = True
        elif kind == "InstEventSemaphore" and str(inst.name).startswith("barrier_"):
            drop = True
        elif kind == "InstMemset":
            out = inst.outs[0]
            ref = getattr(out, "memref", "") or ""
            if str(ref).startswith("const-"):
                drop = True
        if drop:
            removed.append(inst.name)
        else:
            keep.append(inst)
    del main.instructions[:]
    for inst in keep:
        main.add_instruction(inst)
    return removed


def build(strip: bool = True):
    nc = bass.Bass("TRN2", target_bir_lowering=False, debug=False)
    a = nc.dram_tensor("a", [SHARD_ROWS, N_COLS], F32, kind="ExternalInput")
    out_cs = nc.dram_tensor("out_cs", [P, N_COLS], F32, kind="ExternalOutput")
    out_sq = nc.dram_tensor("out_sq", [P, N_STAT], F32, kind="ExternalOutput")

    with ExitStack() as ctx:
        buf = ctx.enter_context(nc.sbuf_tensor("buf", [P, N_TILES, N_COLS], F32))
        x = ctx.enter_context(nc.sbuf_tensor("x", [P, 2, N_COLS], F32))
        f = ctx.enter_context(nc.sbuf_tensor("f", [P, N_COLS], F32))
        scr = [
            ctx.enter_context(nc.sbuf_tensor(f"scr{i}", [P, 2 * N_COLS], F32))
            for i in range(NBIG)
        ]
        scr2 = [
            ctx.enter_context(nc.sbuf_tensor(f"scr2_{i}", [P, N_COLS], F32))
            for i in range(2)
        ]
        stats = ctx.enter_context(nc.sbuf_tensor("stats", [P, N_STAT], F32))
        bias = ctx.enter_context(nc.sbuf_tensor("bias", [P, 1], F32))

        dma_sems = [nc.alloc_semaphore(f"dma{c}") for c in range(N_CHUNKS)]
        dve = nc.alloc_semaphore("dve")  # DVE same-engine chain ordering
        sq_done = nc.alloc_semaphore("sq_done")  # +1 per ACT square op
        bias_ok = nc.alloc_semaphore("bias_ok")
        cs_done = nc.alloc_semaphore("cs_done")
        out_done = nc.alloc_semaphore("out_done")

        with nc.Block() as block:

            @block.sync
            def _(sync):
                for c in range(NBIG):
                    src = a[c * 256 : (c + 1) * 256, :].rearrange(
                        "(p t) d -> p t d", p=P
                    )
                    sync.dma_start(out=buf[:, 2 * c : 2 * c + 2, :], in_=src).then_inc(
                        dma_sems[c], 16
                    )
                for k in range(2):
                    r0 = NBIG * 256 + k * P
                    sync.dma_start(
                        out=buf[:, 14 + k, :], in_=a[r0 : r0 + P, :]
                    ).then_inc(dma_sems[NBIG + k], 16)
                sync.wait_ge(cs_done, 1)
                sync.dma_start(out=out_cs.ap(), in_=f[:]).then_inc(out_done, 16)

            @block.vector
            def _(vector):
                # Waits ride on the compute instructions themselves (no
                # standalone wait_ge) to minimize sequencer overhead in the
                # serial chain.
                i = 0
                # X = c0 + c1 (1024-wide). One wait rides on the compute
                # instruction (hardware allows a single on-wait); the rest
                # are standalone sequencer waits.
                vector.wait_ge(dma_sems[0], 16)
                ins = vector.tensor_add(x[:], buf[:, 0:2, :], buf[:, 2:4, :])
                ins._wait_ge(dma_sems[1], 16)
                ins.then_inc(dve, 1)
                i += 1
                # X += c2..c6
                for c in range(2, NBIG):
                    vector.wait_ge(dma_sems[c], 16)
                    ins = vector.tensor_add(x[:], x[:], buf[:, 2 * c : 2 * c + 2, :])
                    ins._wait_ge(dve, i)
                    ins.then_inc(dve, 1)
                    i += 1
                # X += [t14 | t15] (1024-wide)
                vector.wait_ge(dma_sems[7], 16)
                vector.wait_ge(dma_sems[8], 16)
                ins = vector.tensor_add(x[:], x[:], buf[:, 14:16, :])
                ins._wait_ge(dve, i)
                ins.then_inc(dve, 1)
                i += 1
                # fold: F = X[:,0,:] + X[:,1,:] -- the only post-stream op
                ins = vector.tensor_add(f[:], x[:, 0, :], x[:, 1, :])
                ins._wait_ge(dve, i)
                ins.then_inc(cs_done, 1)

            @block.scalar
            def _(scalar):
                scalar.wait_ge(bias_ok, 1)
                n = 0
                for c in range(NBIG):
                    scalar.wait_ge(dma_sems[c], 16)
                    flat = buf[:, 2 * c : 2 * c + 2, :].rearrange("p t d -> p (t d)")
                    scalar.activation(
                        scr[c][:],
                        flat,
                        mybir.ActivationFunctionType.Square,
                        bias=bias[:],
                        accum_out=stats[:, c : c + 1],
                    ).then_inc(sq_done, 1)
                    n += 1
                for k in range(2):
                    scalar.wait_ge(dma_sems[NBIG + k], 16)
                    scalar.activation(
                        scr2[k][:],
                        buf[:, 14 + k, :],
                        mybir.ActivationFunctionType.Square,
                        bias=bias[:],
                        accum_out=stats[:, NBIG + k : NBIG + k + 1],
                    ).then_inc(sq_done, 1)
                    n += 1
                # ACT ships its own result
                scalar.wait_ge(sq_done, n)
                scalar.dma_start(out=out_sq.ap(), in_=stats[:]).then_inc(out_done, 16)

            @block.gpsimd
            def _(gpsimd):
                gpsimd.memset(bias[:], 0.0).then_inc(bias_ok, 1)
                gpsimd.wait_ge(out_done, 32)

        # No in-kernel sem clear: the NEFF epilogue zeroes every semaphore
        # after the final all-engine barrier (which the Block exit emits, and
        # which orders that teardown after the output DMAs have landed).

    if strip:
        _strip_entry_overhead(nc)
    return nc


_nc_cache = None

# Set by kernel() after each run; test harnesses can read exec_time_ns etc.
LAST_RESULTS = None


def _get_nc():
    global _nc_cache
    if _nc_cache is None:
        _nc_cache = build()
    return _nc_cache


def kernel(A: np.ndarray) -> np.ndarray:
    global LAST_RESULTS
    a = np.ascontiguousarray(np.asarray(A, dtype=np.float32))
    assert a.shape == (N_ROWS, N_COLS), a.shape

    nc = _get_nc()
    shards = a.reshape(N_CORES, SHARD_ROWS, N_COLS)
    in_maps = [{"a": np.ascontiguousarray(shards[c])} for c in range(N_CORES)]
    results = run_bass_kernel_spmd(nc, in_maps, list(range(N_CORES)))
    LAST_RESULTS = results

    cs = np.zeros(N_COLS, dtype=np.float64)
    sq = 0.0
    for r in results.results:
        cs += r["out_cs"].astype(np.float64).sum(axis=0)
        sq += float(r["out_sq"].astype(np.float64).sum())
    total = float(cs @ cs)
    denom = float(N_ROWS) * float(N_ROWS - 1)
    return np.asarray((total - sq) / denom, dtype=np.float32)



# revision 15
# speedup vs baseline: 1.1857x; 1.1857x over previous
"""Trainium2 Bass kernel for nn_DDC2Loss: mean of strict-upper-triangle of A@A.T.

Identity: sum_{i<j} <a_i,a_j> = (||colsum(A)||^2 - sum(A*A)) / 2, so each of
8 row-shards only produces a column-sum partial and sum-of-squares partials;
the tiny final combine runs on host in float64.

v2 design (vs baseline): three-way engine split so every arriving column is
touched once per reduction at line rate:
  - TensorE: the ENTIRE column-sum as ones-weight float32r matmuls (1 cyc/row
    at N=512) accumulating into PSUM [1,512]; result ships as [1,512] (2KB)
    instead of [128,512] (256KB). fp32r rounds inputs tf32-ish, which costs
    ~1e-4 relative on the colsum - far inside the 2e-2 budget.
  - ACT: Square+accum_out on alternating DMA chunks (act table preloaded via
    a dummy square at kernel start, off the critical path).
  - DVE: scalar_tensor_tensor (out=(x*1)*x, accum_out=rowsum) on the other
    chunks, one fused pass per chunk. (tensor_tensor_reduce and gpsimd
    scalar_tensor_tensor both fail walrus codegen - "ISA wrong length".)
DMA: 2-tile chunks (one 128-descriptor issue each) issued back-to-back from
SP on one HWDGE ring; completions are FIFO so ONE semaphore with cumulative
thresholds gates per-chunk compute. Small tail chunks shrink the drain.
"""

import os
import sys

import numpy as np

for _p in (
    "/root/.axon_site",
    "/root/.axon_site/_ro/trn_rl_repo",
    "/root/.axon_site/_ro/pypackages",
    "/opt/trn_rl_repo",
):
    if os.path.isdir(_p) and _p not in sys.path:
        sys.path.append(_p)

from concourse.bass_utils import run_bass_kernel_spmd


def _install_ntff_shim():
    """This image's antenv lacks axon_hooks, but bass_utils imports it when
    BASS_TRACE is set. Synthesize the module (wired to the ctypes NTFF
    profiler from trn_agent_boot when available) so tracing works instead
    of crashing."""
    import types

    if "antenv.axon_hooks" in sys.modules:
        return
    try:
        import antenv  # noqa: F401
    except Exception:
        return
    if getattr(antenv, "axon_hooks", None) is not None:
        return
    mod = types.ModuleType("antenv.axon_hooks")
    mod._hook = None

    def set_axon_ntff_profile_hook(h):
        mod._hook = h

    def get_axon_ntff_profile_hook():
        return mod._hook

    mod.set_axon_ntff_profile_hook = set_axon_ntff_profile_hook
    mod.get_axon_ntff_profile_hook = get_axon_ntff_profile_hook
    sys.modules["antenv.axon_hooks"] = mod
    antenv.axon_hooks = mod
    try:
        from trn_agent_boot.trn_boot import _ntff_profile_via_ctypes

        so = "/opt/axon/libaxon_pjrt.so"
        if os.path.exists(so):
            mod._hook = _ntff_profile_via_ctypes(so)
        import concourse.bass_utils as _bu

        _orig_upload = _bu.upload_artifacts

        def _safe_upload(tmpdir):
            try:
                return _orig_upload(tmpdir)
            except Exception:
                return tmpdir

        _bu.upload_artifacts = _safe_upload
    except Exception:
        pass


_install_ntff_shim()

from contextlib import ExitStack

import concourse.bass as bass
import concourse.mybir as mybir

N_CORES = 8
N_ROWS = 16384
N_COLS = 512
SHARD_ROWS = N_ROWS // N_CORES  # 2048
P = 128
N_TILES = SHARD_ROWS // P  # 16

F32 = mybir.dt.float32
F32R = mybir.dt.float32r
U32 = mybir.dt.uint32

# ---- tunable schedule ------------------------------------------------------
# DMA chunk sizes in tiles (tile = [128 x 512] = 256KB).
CHUNKS = [2, 2, 2, 2, 2, 2, 2, 1, 1]
# Square-work segments: (engine, tile0, ntiles) over the global 16-tile view.
# "A"=ACT Square+accum, "D"=DVE scalar_tensor_tensor. Must tile [0,16).
SEGS = [
    ("A", 0, 2),  # chunk 0
    ("D", 2, 2),  # chunk 1
    ("A", 4, 2),  # chunk 2
    ("D", 6, 2),  # chunk 3
    ("A", 8, 2),  # chunk 4
    ("D", 10, 2),  # chunk 5
    ("A", 12, 2),  # chunk 6
    ("D", 14, 1),  # chunk 7
    ("D", 15, 1),  # chunk 8
]
assert sum(CHUNKS) == N_TILES
_cov = sorted((t0, t0 + n) for _, t0, n in SEGS)
assert _cov[0][0] == 0 and _cov[-1][1] == N_TILES
for (_, e0), (s1, _) in zip(_cov, _cov[1:]):
    assert e0 == s1, "segments must tile the shard"

N_ACT = sum(1 for e, _, _ in SEGS if e == "A")
N_DVE = sum(1 for e, _, _ in SEGS if e == "D")
N_STAT = N_ACT + N_DVE
MAX_SEG = max(n for _, _, n in SEGS) * N_COLS

# chunk boundaries
_ends = []
_o = 0
for _t in CHUNKS:
    _o += _t
    _ends.append(_o)


def _chunk_of(t0, n):
    """Index of the chunk whose completion proves tiles [t0, t0+n) landed.
    (Completions are NOT FIFO across DMAs - the 16 SDMA engines progress
    independently - so each chunk gets its own semaphore.)"""
    last = t0 + n
    for i, e in enumerate(_ends):
        if e >= last:
            return i
    raise AssertionError


def _strip_entry_overhead(nc):
    """Remove const-AP memsets and the entry all-engine barrier from the
    first block; this kernel uses neither (bias is an explicit tile)."""
    main = nc.m.functions[0].blocks[0]
    keep = []
    removed = []
    for inst in main.instructions:
        kind = type(inst).__name__
        drop = False
        if kind == "InstDrain":
            drop = True
        elif kind == "InstRegisterMove":
            drop = True
        elif kind == "InstEventSemaphore" and str(inst.name).startswith("barrier_"):
            drop = True
        elif kind == "InstMemset":
            out = inst.outs[0]
            ref = getattr(out, "memref", "") or ""
            if str(ref).startswith("const-"):
                drop = True
        if drop:
            removed.append(inst.name)
        else:
            keep.append(inst)
    del main.instructions[:]
    for inst in keep:
        main.add_instruction(inst)
    return removed


def build(strip: bool = True):
    nc = bass.Bass("TRN2", target_bir_lowering=False, debug=False)
    # a/buf/ones are float32r (bit-identical to fp32) so the BIR verifier
    # accepts them as fp32r-matmul operands; square ops bitcast back to f32.
    a = nc.dram_tensor("a", [SHARD_ROWS, N_COLS], F32R, kind="ExternalInput")
    out_cs = nc.dram_tensor("out_cs", [1, N_COLS], F32, kind="ExternalOutput")
    out_sq = nc.dram_tensor("out_sq", [P, N_STAT], F32, kind="ExternalOutput")

    offs = []
    o = 0
    for t in CHUNKS:
        offs.append(o)
        o += t

    with ExitStack() as ctx:
        buf = ctx.enter_context(nc.sbuf_tensor("buf", [P, N_TILES, N_COLS], F32R))
        scr_a = ctx.enter_context(nc.sbuf_tensor("scr_a", [P, MAX_SEG], F32))
        scr_d = ctx.enter_context(nc.sbuf_tensor("scr_d", [P, MAX_SEG], F32))
        stats = ctx.enter_context(nc.sbuf_tensor("stats", [P, N_STAT], F32))
        ones = ctx.enter_context(nc.sbuf_tensor("ones", [P, 1], F32R))
        bias = ctx.enter_context(nc.sbuf_tensor("bias", [P, 1], F32))
        cs_sb = ctx.enter_context(nc.sbuf_tensor("cs_sb", [1, N_COLS], F32))
        psum = ctx.enter_context(nc.psum_tensor("cs_ps", [1, N_COLS], F32))

        din = [nc.alloc_semaphore(f"din{c}") for c in range(len(CHUNKS))]
        ready = nc.alloc_semaphore("ready")  # ones/bias seeded
        mm_done = nc.alloc_semaphore("mm_done")  # PE accumulation finished
        dve_done = nc.alloc_semaphore("dve_done")  # DVE squares finished
        cs_ready = nc.alloc_semaphore("cs_ready")  # PSUM->SBUF copy landed
        out_done = nc.alloc_semaphore("out_done")  # both output DMAs landed

        def seg_ap(t0, n):
            """[128, n*512] f32 view of tiles [t0, t0+n)."""
            return (
                buf[:, t0 : t0 + n, :]
                .rearrange("p t d -> p (t d)")
                .bitcast(F32)
            )

        # stat slot assignment: ACT slots first, then DVE
        slot_iter = {"A": 0, "D": N_ACT}

        def next_slot(e):
            s = slot_iter[e]
            slot_iter[e] += 1
            return stats[:, s : s + 1]

        with nc.Block() as block:

            @block.sync
            def _(sync):
                for c, T in enumerate(CHUNKS):
                    r0 = offs[c] * P
                    src = a[r0 : r0 + T * P, :].rearrange("(p t) d -> p t d", p=P)
                    sync.dma_start(
                        out=buf[:, offs[c] : offs[c] + T, :], in_=src
                    ).then_inc(din[c], 16)
                sync.wait_ge(cs_ready, 1)
                sync.dma_start(out=out_cs.ap(), in_=cs_sb[:]).then_inc(out_done, 16)

            @block.tensor
            def _(tensor):
                tensor.wait_ge(ready, 1)
                n_mm = 0
                for c, T in enumerate(CHUNKS):
                    tensor.wait_ge(din[c], 16)
                    for j in range(T):
                        ins = tensor.matmul(
                            out=psum[:],
                            lhsT=ones[:],
                            rhs=buf[:, offs[c] + j, :],
                            start=(n_mm == 0),
                            stop=(n_mm == N_TILES - 1),
                        )
                        n_mm += 1
                ins.then_inc(mm_done, 1)

            @block.scalar
            def _(scalar):
                # dummy square: pulls the act-table load off the critical path
                scalar.wait_ge(ready, 1)
                scalar.activation(
                    scr_a[:, 0:1],
                    bias[:],
                    mybir.ActivationFunctionType.Square,
                    bias=bias[:],
                )
                for e, t0, n in SEGS:
                    if e != "A":
                        continue
                    scalar.wait_ge(din[_chunk_of(t0, n)], 16)
                    scalar.activation(
                        scr_a[:, : n * N_COLS],
                        seg_ap(t0, n),
                        mybir.ActivationFunctionType.Square,
                        bias=bias[:],
                        accum_out=next_slot("A"),
                    )
                # ship the squares once DVE partials are in
                scalar.wait_ge(dve_done, 1)
                scalar.dma_start(out=out_sq.ap(), in_=stats[:]).then_inc(out_done, 16)

            @block.vector
            def _(vector):
                for e, t0, n in SEGS:
                    if e != "D":
                        continue
                    ap = seg_ap(t0, n)
                    ins = vector.scalar_tensor_tensor(
                        out=scr_d[:, : n * N_COLS],
                        in0=ap,
                        scalar=1.0,
                        in1=ap,
                        op0=mybir.AluOpType.mult,
                        op1=mybir.AluOpType.mult,
                        accum_out=next_slot("D"),
                    )
                    ins._wait_ge(din[_chunk_of(t0, n)], 16)
                ins.then_inc(dve_done, 1)
                # colsum PSUM -> SBUF, then SP ships it
                ins = vector.tensor_copy(cs_sb[:], psum[:])
                ins._wait_ge(mm_done, 1)
                ins.then_inc(cs_ready, 1)

            @block.gpsimd
            def _(gpsimd):
                # memset of an f32r-typed AP fails walrus codegen; write the
                # 1.0f bit pattern through a uint32 view instead.
                gpsimd.memset(ones[:].bitcast(U32), 0x3F800000)
                gpsimd.memset(bias[:], 0.0).then_inc(ready, 1)
                gpsimd.wait_ge(out_done, 32)

    if strip:
        _strip_entry_overhead(nc)
    return nc


_nc_cache = None

# Set by kernel() after each run; test harnesses can read exec_time_ns etc.
LAST_RESULTS = None


def _get_nc():
    global _nc_cache
    if _nc_cache is None:
        _nc_cache = build()
    return _nc_cache


def kernel(A: np.ndarray) -> np.ndarray:
    global LAST_RESULTS
    a = np.ascontiguousarray(np.asarray(A, dtype=np.float32))
    assert a.shape == (N_ROWS, N_COLS), a.shape

    nc = _get_nc()
    shards = a.reshape(N_CORES, SHARD_ROWS, N_COLS)
    in_maps = [{"a": np.ascontiguousarray(shards[c])} for c in range(N_CORES)]
    results = run_bass_kernel_spmd(nc, in_maps, list(range(N_CORES)))
    LAST_RESULTS = results

    cs = np.zeros(N_COLS, dtype=np.float64)
    sq = 0.0
    for r in results.results:
        cs += r["out_cs"].astype(np.float64).reshape(-1)
        sq += float(r["out_sq"].astype(np.float64).sum())
    total = float(cs @ cs)
    denom = float(N_ROWS) * float(N_ROWS - 1)
    return np.asarray((total - sq) / denom, dtype=np.float32)
